# Initial kernel scaffold
#
"""Binary conv2d (XNOR-style) + per-channel scale for Trainium2 — v1.

y = conv2d(sign(x), sign(w), stride=1, pad=1) * scale[oc]

Data-parallel over batch across 8 NeuronCores (4 images each).  vs the
previous version:
  - weights are sign()ed and laid out on the HOST and shipped as fp8
    (+-1), removing 10 DMAs and 9 ACT sign ops from the startup path;
  - x binarization: ACT `sign` activation for odd slots, DVE 2-pass
    clamp(x*1e38, -1, 1) for even slots (native ALU ops only --
    is_ge/subtract lowers to DVE/Pool MICROCODE at ~15ns/elem, 20x
    slower than the native path);
  - input loads are paced by the xin pool rotation (bufs=2): image
    n+1's load cannot start before image n's sign released the buffer,
    so image 0's load gets the full HBM bandwidth at startup (the Tile
    scheduler reorders dependency-free DMAs ahead of emission order).
The 3x3 conv over 256 in-channels is accumulating fp8 DoubleRow
matmuls (K=256) into a PSUM tile per 8-output-row chunk, using shifted
windows of a zero-padded 57-column-stride image.  PSUM accumulates in
fp32 and all matmul inputs are exactly representable, so the result is
bit-identical to the fp32 reference.
"""

import numpy as np
import ml_dtypes

N_CORES = 8
IMGS = 4  # images per core
IC = 256
OC = 256
H = W = 56
# Padded row stride is 57, not 58: for a 3-wide kernel the left pad of
# row r+1 doubles as the right pad of row r, halving the dead columns.
WPAD = 57
XPAD_F = 3312  # 58 padded rows * 57 = 3306 -> pad to mult of 16
ROWS = 8  # output rows per PSUM tile
NFREE = ROWS * WPAD  # 456 <= 512 (PSUM bank limit)
NCHUNK = H // ROWS  # 7

_cache = {}


def _install_drain_patch():
    """This walrus build rejects >1 sync-wait on ctrl-type instructions;
    Tile's kernel-tail drain carries one wait per pending proc.  Split it
    into one drain per proc (each with <=1 wait)."""
    import concourse.tile as _tile
    from concourse.vector_clock import ScopedClock, VectorClock

    if getattr(_tile.TileContext, "_drain_split_patch", False):
        return

    def _drain_and_barrier(self, tick_clock, wait_clock):
        nc = self.nc
        gclock = tick_clock.global_clock
        n = len(gclock)
        for p in range(n):
            t = gclock[p]
            if t <= 0:
                continue
            vec = [0] * n
            vec[p] = t
            d = nc.gpsimd.drain()
            wait_clock.add_sem_waits(d.ins, ScopedClock({None: VectorClock(vec)}))
        assert self.sems is not None
        popped = nc._tile_sem_poison_stack.pop()
        assert popped is self._sem_poison
        nc.clear_and_free_semaphores(list(self.sems.allocated().values()))

    _tile.TileContext._drain_and_barrier = _drain_and_barrier
    _tile.TileContext._drain_split_patch = True


def _split_excess_waits(nc, maxw=1):
    """Same walrus limitation: hoist excess sync-waits onto same-engine
    NoOps inserted just before the instruction (engine streams are
    in-order, so a preceding NoOp carrying the waits is equivalent)."""
    import concourse.mybir as mybir

    n_split = 0
    for f in nc.m.functions:
        for bb in f.blocks:
            out = []
            for ins in bb.instructions:
                si = ins.sync_info
                if si and si.on_wait and len(si.on_wait) > maxw:
                    waits = list(si.on_wait)
                    excess, keep = waits[:-maxw], waits[-maxw:]
                    for i in range(0, len(excess), maxw):
                        nop = mybir.InstNoOp(
                            name=f"{ins.name}_waitsplit{i}",
                            engine=ins.engine,
                            ins=[],
                            outs=[],
                            sync_info=mybir.SyncInfo(
                                on_wait=excess[i : i + maxw], on_update=[]
                            ),
                        )
                        out.append(nop)
                    si.on_wait = keep
                    n_split += 1
                out.append(ins)
            bb.instructions = out
    return n_split


def build_nc():
    import concourse.bass as bass
    import concourse.mybir as mybir
    from concourse.tile import TileContext

    _install_drain_patch()

    f32 = mybir.dt.float32
    fp8 = mybir.dt.float8e4
    DR = mybir.MatmulPerfMode.DoubleRow
    Copy = mybir.ActivationFunctionType.Copy

    nc = bass.Bass()
    x = nc.declare_dram_parameter("x", [IMGS, IC, H, W], f32, isOutput=False)
    wb8 = nc.declare_dram_parameter("wb8", [128, 18, OC], fp8, isOutput=False)
    sc2 = nc.declare_dram_parameter("sc2", [128, 2], f32, isOutput=False)
    y = nc.declare_dram_parameter("y", [IMGS, OC, H, W], f32, isOutput=True)

    with TileContext(nc) as tc:
        with (
            tc.tile_pool(name="const", bufs=1) as cpool,
            tc.tile_pool(name="xin", bufs=2) as xin_pool,
            tc.tile_pool(name="outp", bufs=6) as out_pool,
            tc.tile_pool(name="psum", bufs=8, space="PSUM") as psum_pool,
        ):
            wb = cpool.tile([128, 18, OC], fp8)
            sc = cpool.tile([128, 2], f32)
            xp = cpool.tile([128, IMGS * 2, XPAD_F], fp8)

            # --- startup DMAs.  The DMA pool delivers transfers roughly in
            # global issue order, so sequence them by NEED: both slots'
            # band-0 first (unblocks the band-0 signs), then the first
            # weight taps (tiny; unblock LDWEIGHTS), then band 1 and the
            # remaining taps.  Ring k-th issues pair up across sync/scalar.
            HB = H // 2
            xin0 = xin_pool.tile([128, H, W], f32, name="xin0", tag="xin")
            xin1 = xin_pool.tile([128, H, W], f32, name="xin1", tag="xin")
            nc.sync.dma_start(out=xin0[:, 0:HB, :], in_=x[0, 0:128, 0:HB, :])
            nc.scalar.dma_start(out=xin1[:, 0:HB, :], in_=x[0, 128:256, 0:HB, :])
            nc.sync.dma_start(out=wb[:, 0:4, :], in_=wb8[:, 0:4, :])
            nc.scalar.dma_start(out=wb[:, 4:8, :], in_=wb8[:, 4:8, :])
            nc.sync.dma_start(out=xin0[:, HB:H, :], in_=x[0, 0:128, HB:H, :])
            nc.scalar.dma_start(out=wb[:, 8:18, :], in_=wb8[:, 8:18, :])
            nc.sync.dma_start(out=xin1[:, HB:H, :], in_=x[0, 128:256, HB:H, :])
            nc.scalar.dma_start(out=sc[:], in_=sc2[:, :])
            xins = {0: xin0, 1: xin1}

            # --- PE clock warmup.  The tensor engine ramps 1.2 -> 2.4 GHz
            # only after ~3us of CONTINUOUS execution, so without this the
            # first ~13 real matmuls run at half clock (380 ns).  Keep the
            # PE busy on throwaway DoubleRow matmuls over a zeroed scratch
            # tile (no data deps) for the whole startup-delivery window so
            # the real stream starts at full clock.
            scr = cpool.tile([128, 2, 592], fp8)  # ktile step %16 == 0
            nc.gpsimd.memset(scr[:], 0.0)
            for k in range(48):
                ps = psum_pool.tile([128, NFREE], f32, name=f"warm{k}", tag="ps")
                nc.tensor.matmul(
                    ps[:], scr[:, :, 456:584], scr[:, :, 0:NFREE],
                    start=True, stop=True, perf_mode=DR,
                )

            def pad_ring(j):
                # zero only the padding ring (interior is overwritten by
                # the sign): top pad row; each data row's col 0 (also the
                # previous row's right pad); bottom pad row + tail.
                eng = nc.vector if j % 2 == 0 else nc.gpsimd
                xpj = xp[:, j, :]
                eng.memset(xpj[:, 0:WPAD], 0.0)
                lefts = xpj[:, WPAD : WPAD + H * WPAD].rearrange(
                    "p (r c) -> p r c", c=WPAD
                )[:, :, 0:1]
                eng.memset(lefts, 0.0)
                eng.memset(xpj[:, (H + 1) * WPAD : XPAD_F], 0.0)

            def sign_dst(j, r0=0, r1=H):
                base = (r0 + 1) * WPAD + 1
                return (
                    xp[:, j, base : base + (r1 - r0) * WPAD]
                    .rearrange("p (h w) -> p h w", w=WPAD)[:, :, 0:W]
                )

            def sign_slot(j, xin, r0=0, r1=H):
                # binarize to +-1 via the ACT sign activation.  ALL signs
                # live on ACT and ALL drains on DVE so an image's sign can
                # never queue behind the previous image's drain burst
                # (head-of-line blocking exposed by the warm PE start).
                nc.scalar.sign(sign_dst(j, r0, r1), xin[:, r0:r1, :])

            # image-0 pads + banded signs, band-0s of BOTH slots first so
            # the first matmul isn't blocked behind slot0's band 1.
            pad_ring(0)
            pad_ring(1)
            for r0, r1 in ((0, HB), (HB, H)):
                for j in (0, 1):
                    sign_slot(j, xins[j], r0, r1)

            def load_image(n):
                # input loads ride the Scalar-engine HWDGE ring; they are
                # emitted between compute groups so the issue (and thus the
                # transfer) is paced behind the previous image's drains and
                # does not steal DMA bandwidth from image 0's startup load.
                for icb in range(2):
                    j = n * 2 + icb
                    xin = xin_pool.tile([128, H, W], f32, name=f"xin{j}", tag="xin")
                    nc.scalar.dma_start(
                        out=xin[:], in_=x[n, icb * 128 : (icb + 1) * 128, :, :]
                    )
                    pad_ring(j)
                    sign_slot(j, xin)

            drain_rr = [0]

            def compute_image(n, subs=((0, NCHUNK),)):
                # tap-outer (weight-stationary) so consecutive matmuls hit
                # different PSUM banks.  LDWEIGHTS overlaps MATMUL via the
                # PE dual weight buffer.
                # flat 456-wide rhs windows: a 3-dim [rows, cols] rhs AP
                # measures ~47 ns/matmul SLOWER in the accumulating kernel
                # despite covering 8 fewer elements, so keep the flat AP.
                for c0, c1 in subs:
                    for ocb in range(2):
                        psums = [
                            psum_pool.tile(
                                [128, NFREE], f32, name=f"ps{n}{ocb}{c}", tag="ps"
                            )
                            for c in range(c0, c1)
                        ]
                        for t in range(9):
                            kh, kw = divmod(t, 3)
                            lhsT = wb[:, 2 * t : 2 * t + 2, ocb * 128 : (ocb + 1) * 128]
                            rhs_slot = xp[:, 2 * n : 2 * n + 2, :]
                            for c in range(c0, c1):
                                off = c * ROWS * WPAD + kh * WPAD + kw
                                nc.tensor.matmul(
                                    psums[c - c0][:],
                                    lhsT,
                                    rhs_slot[:, :, off : off + NFREE],
                                    start=(t == 0),
                                    stop=(t == 8),
                                    perf_mode=DR,
                                )
                        for c in range(c0, c1):
                            out_c = out_pool.tile([128, ROWS, W], f32)
                            src = psums[c - c0].rearrange("p (h w) -> p h w", w=WPAD)[
                                :, :, 0:W
                            ]
                            # all drains on DVE (signs own ACT; Pool cannot
                            # read PSUM); the fp32 scale is applied here
                            nc.vector.tensor_scalar_mul(
                                out_c[:], src, sc[:, ocb : ocb + 1]
                            )
                            nc.sync.dma_start(
                                out=y[n, ocb * 128 : (ocb + 1) * 128, c * ROWS : (c + 1) * ROWS, :],
                                in_=out_c[:],
                            )

            # interleave: image n+1's loads/signs are emitted between image
            # n's compute subgroups so their DMA issues are paced behind the
            # first drains (keeping startup DMA bandwidth for image 0).
            compute_image(0, subs=((0, 3),))
            load_image(1)
            compute_image(0, subs=((3, NCHUNK),))
            compute_image(1, subs=((0, 4),))
            load_image(2)
            compute_image(1, subs=((4, NCHUNK),))
            compute_image(2, subs=((0, 4),))
            load_image(3)
            compute_image(2, subs=((4, NCHUNK),))
            compute_image(3, subs=((0, 3), (3, 5), (5, NCHUNK)))

    _split_excess_waits(nc)
    return nc


def _get_nc():
    if "nc" not in _cache:
        _cache["nc"] = build_nc()
    return _cache["nc"]


def _prep_weights(weight, scale):
    # host-side: binarize weights, lay out [p, (kh kw icb), oc] fp8; the
    # per-channel scale is rearranged to [p, ocb].
    w = np.asarray(weight, dtype=np.float32)  # [oc, ic, kh, kw]
    wb = np.sign(w).transpose(2, 3, 1, 0)  # [kh, kw, ic, oc]
    wb = wb.reshape(3, 3, 2, 128, OC).transpose(3, 0, 1, 2, 4).reshape(128, 18, OC)
    wb8 = np.ascontiguousarray(wb).astype(ml_dtypes.float8_e4m3)
    sc2 = np.ascontiguousarray(np.asarray(scale, dtype=np.float32).reshape(2, 128).T)
    return wb8, sc2


def run(inputs, trace=False, trace_cores=None):
    from concourse.bass_utils import run_bass_kernel_spmd

    x = np.asarray(inputs["x"])
    wb8, sc2 = _prep_weights(inputs["weight"], inputs["scale"])

    in_maps = [
        {"x": x[i * IMGS : (i + 1) * IMGS], "wb8": wb8, "sc2": sc2}
        for i in range(N_CORES)
    ]
    res = run_bass_kernel_spmd(
        _get_nc(),
        in_maps,
        core_ids=list(range(N_CORES)),
        trace=trace,
        trace_cores=trace_cores,
    )
    out = np.concatenate([res.results[i]["y"] for i in range(N_CORES)], axis=0)
    return out, res


def kernel(**inputs):
    # One retry: a previously crashed process can leave a core wedged
    # (NRT_EXEC_UNIT_UNRECOVERABLE); the runtime recovers on the next
    # attempt.
    try:
        out, _ = run(inputs, trace=False)
    except Exception:
        out, _ = run(inputs, trace=False)
    return out



# revision 1
# speedup vs baseline: 1.0368x; 1.0368x over previous
"""Binary conv2d (XNOR-style) + per-channel scale for Trainium2 — v1.

y = conv2d(sign(x), sign(w), stride=1, pad=1) * scale[oc]

Data-parallel over batch across 8 NeuronCores (4 images each).  vs the
previous version:
  - weights are sign()ed and laid out on the HOST and shipped as fp8
    (+-1), removing 10 DMAs and 9 ACT sign ops from the startup path;
  - x binarization: ACT `sign` activation for odd slots, DVE 2-pass
    clamp(x*1e38, -1, 1) for even slots (native ALU ops only --
    is_ge/subtract lowers to DVE/Pool MICROCODE at ~15ns/elem, 20x
    slower than the native path);
  - input loads are paced by the xin pool rotation (bufs=2): image
    n+1's load cannot start before image n's sign released the buffer,
    so image 0's load gets the full HBM bandwidth at startup (the Tile
    scheduler reorders dependency-free DMAs ahead of emission order).
The 3x3 conv over 256 in-channels is accumulating fp8 DoubleRow
matmuls (K=256) into a PSUM tile per 8-output-row chunk, using shifted
windows of a zero-padded 57-column-stride image.  PSUM accumulates in
fp32 and all matmul inputs are exactly representable, so the result is
bit-identical to the fp32 reference.
"""

import numpy as np
import ml_dtypes

N_CORES = 8
IMGS = 4  # images per core
IC = 256
OC = 256
H = W = 56
# Padded row stride is 57, not 58: for a 3-wide kernel the left pad of
# row r+1 doubles as the right pad of row r, halving the dead columns.
WPAD = 57
XPAD_F = 3312  # 58 padded rows * 57 = 3306 -> pad to mult of 16
ROWS = 8  # output rows per PSUM tile
NFREE = ROWS * WPAD  # 456 <= 512 (PSUM bank limit)
NCHUNK = H // ROWS  # 7

_cache = {}


def _install_drain_patch():
    """This walrus build rejects >1 sync-wait on ctrl-type instructions;
    Tile's kernel-tail drain carries one wait per pending proc.  Split it
    into one drain per proc (each with <=1 wait)."""
    import concourse.tile as _tile
    from concourse.vector_clock import ScopedClock, VectorClock

    if getattr(_tile.TileContext, "_drain_split_patch", False):
        return

    def _drain_and_barrier(self, tick_clock, wait_clock):
        nc = self.nc
        gclock = tick_clock.global_clock
        n = len(gclock)
        for p in range(n):
            t = gclock[p]
            if t <= 0:
                continue
            vec = [0] * n
            vec[p] = t
            d = nc.gpsimd.drain()
            wait_clock.add_sem_waits(d.ins, ScopedClock({None: VectorClock(vec)}))
        assert self.sems is not None
        popped = nc._tile_sem_poison_stack.pop()
        assert popped is self._sem_poison
        nc.clear_and_free_semaphores(list(self.sems.allocated().values()))

    _tile.TileContext._drain_and_barrier = _drain_and_barrier
    _tile.TileContext._drain_split_patch = True


def _split_excess_waits(nc, maxw=1):
    """Same walrus limitation: hoist excess sync-waits onto same-engine
    NoOps inserted just before the instruction (engine streams are
    in-order, so a preceding NoOp carrying the waits is equivalent)."""
    import concourse.mybir as mybir

    n_split = 0
    for f in nc.m.functions:
        for bb in f.blocks:
            out = []
            for ins in bb.instructions:
                si = ins.sync_info
                if si and si.on_wait and len(si.on_wait) > maxw:
                    waits = list(si.on_wait)
                    excess, keep = waits[:-maxw], waits[-maxw:]
                    for i in range(0, len(excess), maxw):
                        nop = mybir.InstNoOp(
                            name=f"{ins.name}_waitsplit{i}",
                            engine=ins.engine,
                            ins=[],
                            outs=[],
                            sync_info=mybir.SyncInfo(
                                on_wait=excess[i : i + maxw], on_update=[]
                            ),
                        )
                        out.append(nop)
                    si.on_wait = keep
                    n_split += 1
                out.append(ins)
            bb.instructions = out
    return n_split


def build_nc():
    import concourse.bass as bass
    import concourse.mybir as mybir
    from concourse.tile import TileContext

    _install_drain_patch()

    f32 = mybir.dt.float32
    fp8 = mybir.dt.float8e4
    DR = mybir.MatmulPerfMode.DoubleRow
    Copy = mybir.ActivationFunctionType.Copy

    nc = bass.Bass()
    x = nc.declare_dram_parameter("x", [IMGS, IC, H, W], f32, isOutput=False)
    wb8 = nc.declare_dram_parameter("wb8", [128, 18, OC], fp8, isOutput=False)
    sc2 = nc.declare_dram_parameter("sc2", [128, 2], f32, isOutput=False)
    y = nc.declare_dram_parameter("y", [IMGS, OC, H, W], f32, isOutput=True)

    with TileContext(nc) as tc:
        with (
            tc.tile_pool(name="const", bufs=1) as cpool,
            tc.tile_pool(name="xin", bufs=2) as xin_pool,
            tc.tile_pool(name="outp", bufs=6) as out_pool,
            tc.tile_pool(name="psum", bufs=8, space="PSUM") as psum_pool,
        ):
            wb = cpool.tile([128, 18, OC], fp8)
            sc = cpool.tile([128, 2], f32)
            xp = cpool.tile([128, IMGS * 2, XPAD_F], fp8)

            # --- startup DMAs.  The DMA pool delivers transfers roughly in
            # global issue order, so sequence them by NEED: both slots'
            # band-0 first (unblocks the band-0 signs), then the first
            # weight taps (tiny; unblock LDWEIGHTS), then band 1 and the
            # remaining taps.  Ring k-th issues pair up across sync/scalar.
            HB = H // 2
            xin0 = xin_pool.tile([128, H, W], f32, name="xin0", tag="xin")
            xin1 = xin_pool.tile([128, H, W], f32, name="xin1", tag="xin")
            nc.sync.dma_start(out=xin0[:, 0:HB, :], in_=x[0, 0:128, 0:HB, :])
            nc.scalar.dma_start(out=xin1[:, 0:HB, :], in_=x[0, 128:256, 0:HB, :])
            nc.sync.dma_start(out=wb[:, 0:4, :], in_=wb8[:, 0:4, :])
            nc.scalar.dma_start(out=wb[:, 4:8, :], in_=wb8[:, 4:8, :])
            nc.sync.dma_start(out=xin0[:, HB:H, :], in_=x[0, 0:128, HB:H, :])
            nc.scalar.dma_start(out=wb[:, 8:18, :], in_=wb8[:, 8:18, :])
            nc.sync.dma_start(out=xin1[:, HB:H, :], in_=x[0, 128:256, HB:H, :])
            nc.scalar.dma_start(out=sc[:], in_=sc2[:, :])
            xins = {0: xin0, 1: xin1}

            # --- PE clock warmup.  The tensor engine ramps 1.2 -> 2.4 GHz
            # only after ~3us of CONTINUOUS execution, so without this the
            # first ~13 real matmuls run at half clock (380 ns).  Keep the
            # PE busy on throwaway DoubleRow matmuls over a zeroed scratch
            # tile (no data deps) for the whole startup-delivery window so
            # the real stream starts at full clock.
            scr = cpool.tile([128, 2, 592], fp8)  # ktile step %16 == 0
            nc.gpsimd.memset(scr[:], 0.0)
            for k in range(48):
                ps = psum_pool.tile([128, NFREE], f32, name=f"warm{k}", tag="ps")
                nc.tensor.matmul(
                    ps[:], scr[:, :, 456:584], scr[:, :, 0:NFREE],
                    start=True, stop=True, perf_mode=DR,
                )

            def pad_ring(j):
                # zero only the padding ring (interior is overwritten by
                # the sign): top pad row; each data row's col 0 (also the
                # previous row's right pad); bottom pad row + tail.
                eng = nc.vector if j % 2 == 0 else nc.gpsimd
                xpj = xp[:, j, :]
                eng.memset(xpj[:, 0:WPAD], 0.0)
                lefts = xpj[:, WPAD : WPAD + H * WPAD].rearrange(
                    "p (r c) -> p r c", c=WPAD
                )[:, :, 0:1]
                eng.memset(lefts, 0.0)
                eng.memset(xpj[:, (H + 1) * WPAD : XPAD_F], 0.0)

            def sign_dst(j, r0=0, r1=H):
                base = (r0 + 1) * WPAD + 1
                return (
                    xp[:, j, base : base + (r1 - r0) * WPAD]
                    .rearrange("p (h w) -> p h w", w=WPAD)[:, :, 0:W]
                )

            def sign_slot(j, xin, r0=0, r1=H):
                # binarize to +-1 via the ACT sign activation.  ALL signs
                # live on ACT and ALL drains on DVE so an image's sign can
                # never queue behind the previous image's drain burst
                # (head-of-line blocking exposed by the warm PE start).
                nc.scalar.sign(sign_dst(j, r0, r1), xin[:, r0:r1, :])

            # image-0 pads + banded signs, band-0s of BOTH slots first so
            # the first matmul isn't blocked behind slot0's band 1.
            pad_ring(0)
            pad_ring(1)
            for r0, r1 in ((0, HB), (HB, H)):
                for j in (0, 1):
                    sign_slot(j, xins[j], r0, r1)

            def load_image(n):
                # input loads ride the Scalar-engine HWDGE ring; they are
                # emitted between compute groups so the issue (and thus the
                # transfer) is paced behind the previous image's drains and
                # does not steal DMA bandwidth from image 0's startup load.
                for icb in range(2):
                    j = n * 2 + icb
                    xin = xin_pool.tile([128, H, W], f32, name=f"xin{j}", tag="xin")
                    nc.scalar.dma_start(
                        out=xin[:], in_=x[n, icb * 128 : (icb + 1) * 128, :, :]
                    )
                    pad_ring(j)
                    sign_slot(j, xin)

            drain_rr = [0]

            def compute_image(n, subs=((0, NCHUNK),)):
                # tap-outer (weight-stationary) so consecutive matmuls hit
                # different PSUM banks.  LDWEIGHTS overlaps MATMUL via the
                # PE dual weight buffer.
                # flat 456-wide rhs windows: a 3-dim [rows, cols] rhs AP
                # measures ~47 ns/matmul SLOWER in the accumulating kernel
                # despite covering 8 fewer elements, so keep the flat AP.
                for c0, c1 in subs:
                    for ocb in range(2):
                        psums = [
                            psum_pool.tile(
                                [128, NFREE], f32, name=f"ps{n}{ocb}{c}", tag="ps"
                            )
                            for c in range(c0, c1)
                        ]
                        for t in range(9):
                            kh, kw = divmod(t, 3)
                            lhsT = wb[:, 2 * t : 2 * t + 2, ocb * 128 : (ocb + 1) * 128]
                            rhs_slot = xp[:, 2 * n : 2 * n + 2, :]
                            for c in range(c0, c1):
                                off = c * ROWS * WPAD + kh * WPAD + kw
                                nc.tensor.matmul(
                                    psums[c - c0][:],
                                    lhsT,
                                    rhs_slot[:, :, off : off + NFREE],
                                    start=(t == 0),
                                    stop=(t == 8),
                                    perf_mode=DR,
                                )
                        for c in range(c0, c1):
                            out_c = out_pool.tile([128, ROWS, W], f32)
                            src = psums[c - c0].rearrange("p (h w) -> p h w", w=WPAD)[
                                :, :, 0:W
                            ]
                            # all drains on DVE (signs own ACT; Pool cannot
                            # read PSUM); the fp32 scale is applied here
                            nc.vector.tensor_scalar_mul(
                                out_c[:], src, sc[:, ocb : ocb + 1]
                            )
                            nc.sync.dma_start(
                                out=y[n, ocb * 128 : (ocb + 1) * 128, c * ROWS : (c + 1) * ROWS, :],
                                in_=out_c[:],
                            )

            # interleave: image n+1's loads/signs are emitted between image
            # n's compute subgroups so their DMA issues are paced behind the
            # first drains (keeping startup DMA bandwidth for image 0).
            compute_image(0, subs=((0, 3),))
            load_image(1)
            compute_image(0, subs=((3, NCHUNK),))
            compute_image(1, subs=((0, 4),))
            load_image(2)
            compute_image(1, subs=((4, NCHUNK),))
            compute_image(2, subs=((0, 4),))
            load_image(3)
            compute_image(2, subs=((4, NCHUNK),))
            compute_image(3, subs=((0, 3), (3, 5), (5, NCHUNK)))

    _split_excess_waits(nc)
    return nc


def _get_nc():
    if "nc" not in _cache:
        _cache["nc"] = build_nc()
    return _cache["nc"]


def _prep_weights(weight, scale):
    # host-side: binarize weights, lay out [p, (kh kw icb), oc] fp8; the
    # per-channel scale is rearranged to [p, ocb].
    w = np.asarray(weight, dtype=np.float32)  # [oc, ic, kh, kw]
    wb = np.sign(w).transpose(2, 3, 1, 0)  # [kh, kw, ic, oc]
    wb = wb.reshape(3, 3, 2, 128, OC).transpose(3, 0, 1, 2, 4).reshape(128, 18, OC)
    wb8 = np.ascontiguousarray(wb).astype(ml_dtypes.float8_e4m3)
    sc2 = np.ascontiguousarray(np.asarray(scale, dtype=np.float32).reshape(2, 128).T)
    return wb8, sc2


def run(inputs, trace=False, trace_cores=None):
    from concourse.bass_utils import run_bass_kernel_spmd

    x = np.asarray(inputs["x"])
    wb8, sc2 = _prep_weights(inputs["weight"], inputs["scale"])

    in_maps = [
        {"x": x[i * IMGS : (i + 1) * IMGS], "wb8": wb8, "sc2": sc2}
        for i in range(N_CORES)
    ]
    res = run_bass_kernel_spmd(
        _get_nc(),
        in_maps,
        core_ids=list(range(N_CORES)),
        trace=trace,
        trace_cores=trace_cores,
    )
    out = np.concatenate([res.results[i]["y"] for i in range(N_CORES)], axis=0)
    return out, res


def kernel(**inputs):
    # One retry: a previously crashed process can leave a core wedged
    # (NRT_EXEC_UNIT_UNRECOVERABLE); the runtime recovers on the next
    # attempt.
    try:
        out, _ = run(inputs, trace=False)
    except Exception:
        out, _ = run(inputs, trace=False)
    return out

